# revision 36
# baseline (speedup 1.0000x reference)
"""Trainium2 Bass kernel for nn_DilatedAttention (dynamic per-image 3x3
depthwise filter + affine epilogue), data-parallel over batch on 8 cores.

Math per image (one core):
  pooled[c] = mean_hw(x)                              (64,)
  lf = tanh(BN(pooled @ conv_w.T))                    (72,) = (G=8, k2=9)
  low[c,h,w] = sum_t lf[g(c),t] * x[c, h+di, w+dj]    3x3 reflect-pad conv
  out = A[c]*low + B[c]*x + const[c]
    A = lamb_l*(1+inside_all), B = 1+lamb_h, const = -inside_all*lamb_l*pooled

v7b: hybrid-precision taps, DMA-bandwidth-aware schedule.  The per-core
DMA<->DRAM bandwidth is ~210 GB/s (435 shared by two cores), so the
schedule is built around it: the fp8e4m3 copy of x (4.3 MB) loads FIRST,
split across both HWDGE rings -- it feeds pooling, all eight off-center
taps, and the edge fix-ups -- and the bf16 copy (8.5 MB) streams in
behind it, feeding only the center taps.  Per output tile the 8 off-center
taps run as fp8 PE matmuls (six folded into three DoubleRow pairs; the
256-element row pitch keeps the k-tile byte step %16), then the center tap
(dominant B*x term) joins the same PSUM group in bf16, ordered last so PE
can start before bf16 arrives.  ScalarE evacuates PSUM (+const).  One
6-row FMA tile per 32-row macro-group runs at the macro FRONT on DVE.
Side taps use 255-wide views; the missing reflect column per side is
patched by six batched DVE fix-ups per macro.  SBUF rows keep the native
256 pitch (strided DMA destinations shatter into 512 B descriptors).
All four halo rows (row 0 = reflect/neighbor, row 129 = neighbor/reflect)
come in as direct HBM DMAs -- on-chip reflect copies created scheduler
dependencies on the whole load.  Consts ride the GpSimd SWDGE ring; all
stores ride SP, the final macro split PE-half-first to shorten the tail.
"""

import os
import sys

import numpy as np

for _p in ("/opt/trn_rl_repo",):
    if _p not in sys.path:
        sys.path.insert(0, _p)

import bass_rust
import concourse.bass as bass
import concourse.bacc as bacc
import concourse.mybir as mybir
import concourse.tile as tile
from concourse.bass_utils import run_bass_kernel_spmd

F32 = mybir.dt.float32
BF16 = mybir.dt.bfloat16
F8 = mybir.dt.float8e4
AF = mybir.ActivationFunctionType
ALU = mybir.AluOpType
DR = mybir.MatmulPerfMode.DoubleRow

C, H, W = 64, 256, 256
NCORES = 8
K2 = 9

# cblob column layout (f32, 128 partitions)
CB_PPOOL = 0           # [128, 128]
CB_G72 = 128           # [72, 128]
CB_CWT = 256           # [64, 72]
CB_MASK9 = 328         # [72, 9]
CB_BNS, CB_BNB, CB_AVEC, CB_BVEC, CB_CLVEC = 337, 338, 339, 340, 341
CB_COLS = 342

# DoubleRow tap pairs (tap idx = 3*i + j; pair = same j, rows i0<i1) and
# fp8 singles; center tap 4 runs in bf16 with B folded in.  The j=1 pair
# is full-width and is emitted first (start=True covers the whole bank).
TAP_PAIRS = ((1, 7), (0, 3), (2, 5))
TAP_SINGLES = (6, 8)

LAST_RESULT = {}


def _install_ntff_hook():
    """Register the axon NTFF profile hook (the image's antenv lacks
    axon_hooks; build it from trn_agent_boot's ctypes shim)."""
    import types

    try:
        from antenv.axon_hooks import get_axon_ntff_profile_hook  # noqa: F401
        return
    except ImportError:
        pass
    mod = types.ModuleType("antenv.axon_hooks")
    _h = [None]
    mod.set_axon_ntff_profile_hook = lambda hook: _h.__setitem__(0, hook)
    mod.get_axon_ntff_profile_hook = lambda: _h[0]
    sys.modules["antenv.axon_hooks"] = mod
    import antenv

    antenv.axon_hooks = mod
    try:
        from trn_agent_boot.trn_boot import _ntff_profile_via_ctypes

        mod.set_axon_ntff_profile_hook(
            _ntff_profile_via_ctypes("/opt/axon/libaxon_pjrt.so")
        )
    except Exception as e:  # hook stays None; tracing degrades gracefully
        print("ntff hook install failed:", e)


def _col_rng(j):
    """(out column slice, in column slice) for horizontal tap offset j."""
    if j == 0:
        return slice(1, 256), slice(0, 255)
    if j == 1:
        return slice(0, 256), slice(0, 256)
    return slice(0, 255), slice(1, 256)


def _build_program():
    nc = bacc.Bacc("TRN2", target_bir_lowering=False, debug=False)

    x_d = nc.declare_dram_parameter("x", [C, H, W], BF16, isOutput=False)
    x8_d = nc.declare_dram_parameter("x8", [C, H, W], F8, isOutput=False)
    out_d = nc.declare_dram_parameter("out", [C, H, W], BF16, isOutput=True)
    cb_d = nc.declare_dram_parameter("cblob", [128, CB_COLS], F32, isOutput=False)
    i128_d = nc.declare_dram_parameter("i128", [128, 128], BF16, isOutput=False)
    i128_8_d = nc.declare_dram_parameter("i128_8", [128, 128], F8, isOutput=False)

    with tile.TileContext(nc) as tc:
        with (
            tc.tile_pool(name="xbuf", bufs=1) as xp,
            tc.tile_pool(name="consts", bufs=1) as cp,
            tc.tile_pool(name="diag", bufs=1) as dp,
            tc.tile_pool(name="psum", bufs=7, space=bass.MemorySpace.PSUM) as pp,
            tc.tile_pool(name="stage", bufs=3) as sp,
            tc.tile_pool(name="spsum", bufs=1, space=bass.MemorySpace.PSUM) as pps,
        ):
            # Layout row r: top half (p<64) holds HBM row r-1, bottom half
            # holds 127+r; rows 0/129 are reflect/neighbor halo rows, all
            # loaded straight from HBM (row 0 top = image row 1, row 129
            # bottom = image row 254).
            x_sb = xp.tile([128, 130, 256], BF16)
            x8_sb = xp.tile([128, 130, 256], F8)
            cblob = cp.tile([128, CB_COLS], F32, tag="cblob")
            i128 = cp.tile([128, 128], BF16, tag="i128")
            i128_8 = cp.tile([128, 128], F8, tag="i128_8")

            # SP ring: fp8 top half (+halos) first, then bf16 top half.
            # fp8 goes in three pieces so pooling can start on the first
            # piece ~10us in instead of waiting for a 65-row chunk.
            segs = [(1, 66), (66, 129)]
            segs8 = [(1, 44), (44, 87), (87, 129)]
            for a, b in segs8:
                nc.sync.dma_start(out=x8_sb[0:64, a:b, :],
                                  in_=x8_d[:, a - 1:b - 1, :])
            nc.sync.dma_start(out=x8_sb[0:64, 0:1, :], in_=x8_d[:, 1:2, :])
            nc.sync.dma_start(out=x8_sb[0:64, 129:130, :],
                              in_=x8_d[:, 128:129, :])
            for a, b in segs:
                nc.sync.dma_start(out=x_sb[0:64, a:b, :],
                                  in_=x_d[:, a - 1:b - 1, :])
            nc.sync.dma_start(out=x_sb[0:64, 129:130, :], in_=x_d[:, 128:129, :])
            # ACT ring: fp8 bottom half (+halos) first, then bf16 bottom half
            for a, b in segs8:
                nc.scalar.dma_start(out=x8_sb[64:128, a:b, :],
                                    in_=x8_d[:, 127 + a:127 + b, :])
            nc.scalar.dma_start(out=x8_sb[64:128, 0:1, :],
                                in_=x8_d[:, 127:128, :])
            nc.scalar.dma_start(out=x8_sb[64:128, 129:130, :],
                                in_=x8_d[:, 254:255, :])
            for a, b in segs:
                nc.scalar.dma_start(out=x_sb[64:128, a:b, :],
                                    in_=x_d[:, 127 + a:127 + b, :])
            nc.scalar.dma_start(out=x_sb[64:128, 129:130, :],
                                in_=x_d[:, 254:255, :])
            # GpSimd ring: small consts only
            nc.gpsimd.dma_start(out=cblob[:], in_=cb_d[:])
            nc.gpsimd.dma_start(out=i128[:], in_=i128_d[:])
            nc.gpsimd.dma_start(out=i128_8[:], in_=i128_8_d[:])

            # ---- pooling from fp8 (layout rows 1..128) under the load ----
            # sub-chunks track the fp8 piece boundaries; DVE takes the first
            # and a small last sub, ScalarE the middle and the other small
            # tail, so the finish is gated by arrival (~26us) + ~5us work
            pstat = cp.tile([128, 4], F32, tag="pstat")
            for k, (a, b) in enumerate([(1, 44), (87, 108)]):
                nc.vector.tensor_reduce(
                    out=pstat[:, k:k + 1], in_=x8_sb[:, a:b, :],
                    axis=mybir.AxisListType.XY, op=ALU.add,
                )
            for k, (a, b) in enumerate([(44, 87), (108, 129)]):
                nc.scalar.activation(
                    x8_sb[:, a:b, :], x8_sb[:, a:b, :],
                    AF.Copy, accum_out=pstat[:, 2 + k:3 + k],
                )
            stat = cp.tile([128, 1], F32, tag="stat")
            nc.vector.tensor_reduce(
                out=stat[:], in_=pstat[:], axis=mybir.AxisListType.X, op=ALU.add
            )

            # pooled[p] = (stat[p%64] + stat[64+p%64]) / 65536  (both halves)
            # (the three small psum results share one bank so the main loop
            # gets 7 of the 8 banks)
            ppool = cblob[:, CB_PPOOL:CB_PPOOL + 128]
            wpsum = pps.tile([128, 11], F32, tag="wpsum")
            pooled_ps = wpsum[:, 0:1]
            lf_ps = wpsum[0:72, 1:2]
            w_ps = wpsum[:, 2:2 + K2]
            nc.tensor.matmul(pooled_ps[:], ppool, stat[:])
            pooled = cp.tile([128, 1], F32, tag="pooled")
            nc.scalar.copy(pooled[:], pooled_ps[:])

            # const[p] = CL[p] * pooled[p]
            cvec = cp.tile([128, 1], F32, tag="cvec")
            nc.vector.tensor_scalar_mul(
                cvec[:], pooled[:], cblob[:, CB_CLVEC:CB_CLVEC + 1])

            # lf = tanh(bns * (pooled @ conv_w.T) + bnb)   [72,1]
            nc.tensor.matmul(lf_ps[:], cblob[0:64, CB_CWT:CB_CWT + 72],
                             pooled[0:64, :])
            lf = cp.tile([72, 1], F32, tag="lf")
            nc.scalar.activation(lf[:], lf_ps[:], AF.Tanh,
                                 bias=cblob[0:72, CB_BNB:CB_BNB + 1],
                                 scale=cblob[0:72, CB_BNS:CB_BNS + 1])

            # W0[p,t] = lf[g(p)*9+t]:  lfmat = mask9 * lf ; W0 = g72.T @ lfmat
            lfmat = cp.tile([72, K2], F32, tag="lfmat")
            nc.vector.tensor_scalar_mul(
                lfmat[:], cblob[0:72, CB_MASK9:CB_MASK9 + K2], lf[:])
            nc.tensor.matmul(w_ps[:], cblob[0:72, CB_G72:CB_G72 + 128], lfmat[:])
            # W = A * W0 ; then center tap += B  (folds B*x into the conv)
            wmat = cp.tile([128, K2], F32, tag="wmat")
            nc.scalar.activation(wmat[:], w_ps[:], AF.Copy,
                                 scale=cblob[:, CB_AVEC:CB_AVEC + 1])
            nc.vector.tensor_scalar_add(
                wmat[:, 4:5], wmat[:, 4:5], cblob[:, CB_BVEC:CB_BVEC + 1])

            # stationary matrices: bf16 center diag, fp8 DoubleRow pairs
            # [128, kt=2, 128] and fp8 singles [128, 128]
            dC = dp.tile([128, 128], BF16, tag="dC")
            nc.vector.tensor_scalar_mul(dC[:], i128[:], wmat[:, 4:5])
            d8p = []
            for k, (tA, tB) in enumerate(TAP_PAIRS):
                d = dp.tile([128, 2, 128], F8, tag=f"d8p{k}", name=f"d8p{k}")
                nc.vector.tensor_scalar_mul(d[:, 0, :], i128_8[:],
                                            wmat[:, tA:tA + 1])
                nc.vector.tensor_scalar_mul(d[:, 1, :], i128_8[:],
                                            wmat[:, tB:tB + 1])
                d8p.append(d)
            d8s = {}
            for t in TAP_SINGLES:
                d = dp.tile([128, 128], F8, tag=f"d8s{t}", name=f"d8s{t}")
                nc.vector.tensor_scalar_mul(d[:], i128_8[:], wmat[:, t:t + 1])
                d8s[t] = d

            # ---- main loop: 4 macro-groups of 32 layout rows ----
            # Per macro: one 6-row DVE FMA tile at the FRONT (rows 0..5, all
            # taps from fp8) + 13 PE tiles (rows 6..31).  PE runs tap-major
            # in batches sharing the 5 PSUM banks; fp8 taps first, bf16
            # center last so PE starts before the bf16 stream lands.
            def pair_view(a, i0, i1, j):
                _, ic = _col_rng(j)
                v = x8_sb[:, a + i0:a + i0 + 2, ic]
                vv = v.copy()
                vv.ap = bass_rust.VecI64Pair(
                    [tuple(v.ap[0]), ((i1 - i0) * 256, 2), (256, 2),
                     (1, ic.stop - ic.start)]
                )
                return vv

            def pe_batch(st32, mg, offs):
                pss = []
                for o in offs:
                    pss.append((pp.tile([128, 2, 256], F32, tag="ps",
                                        name=f"ps{mg}_{o}"), o))
                for k, (tA, tB) in enumerate(TAP_PAIRS):
                    i0, i1, j = tA // 3, tB // 3, tA % 3
                    oc, _ = _col_rng(j)
                    for ps, o in pss:
                        a = 32 * mg + o
                        nc.tensor.matmul(ps[:, :, oc], d8p[k][:],
                                         pair_view(a, i0, i1, j),
                                         start=(k == 0), stop=False,
                                         perf_mode=DR)
                for t in TAP_SINGLES:
                    i, j = t // 3, t % 3
                    oc, ic = _col_rng(j)
                    for ps, o in pss:
                        a = 32 * mg + o
                        nc.tensor.matmul(
                            ps[:, :, oc], d8s[t][:],
                            x8_sb[:, a + i:a + i + 2, ic],
                            start=False, stop=False)
                for ps, o in pss:
                    a = 32 * mg + o
                    nc.tensor.matmul(ps[:], dC[:], x_sb[:, a + 1:a + 3, :],
                                     start=False, stop=True)
                for ps, o in pss:
                    nc.scalar.activation(st32[:, o:o + 2, :], ps[:],
                                         AF.Identity, bias=cvec[:])

            def fma_tile(st32, mg, o, rows):
                # center tap + const on ScalarE (bf16); 8 fp8 taps on DVE
                a, b = 32 * mg + o, 32 * mg + o + rows
                nc.scalar.activation(
                    st32[:, o:o + rows, :], x_sb[:, a + 1:b + 1, :],
                    AF.Identity, bias=cvec[:], scale=wmat[:, 4:5],
                )
                for idx in (0, 1, 2, 3, 5, 6, 7, 8):
                    i, j = idx // 3, idx % 3
                    oc, ic = _col_rng(j)
                    nc.vector.scalar_tensor_tensor(
                        st32[:, o:o + rows, oc], x8_sb[:, a + i:b + i, ic],
                        wmat[:, idx:idx + 1], st32[:, o:o + rows, oc],
                        ALU.mult, ALU.add,
                    )

            sts = [sp.tile([128, 32, 256], BF16, tag="st32", name=f"st{m}")
                   for m in range(4)]
            # fma tiles are emitted one macro AHEAD (after the next macro's
            # first PE batch): their ScalarE center passes then never sit in
            # front of the PSUM evacuations in ScalarE's in-order queue
            # waiting for the late bf16 stream (that starved PE for ~14us)
            fma_tile(sts[0], 0, 0, 6)
            for mg in range(4):
                st32 = sts[mg]
                for bi, offs in enumerate(((6, 8, 10, 12), (14, 16, 18, 20),
                                           (22, 24, 26, 28), (30,))):
                    pe_batch(st32, mg, offs)
                    if bi == 0 and mg < 3:
                        fma_tile(sts[mg + 1], mg + 1, 0, 6)
                # reflect edge columns: add the missing side taps (j=0 at
                # out col 0 reads image col 1; j=2 at col 255 reads col 254);
                # the last macro patches per store-half
                for o0, o1 in ((0, 32),) if mg < 3 else ((16, 32), (0, 16)):
                    for i in range(3):
                        rows = slice(32 * mg + o0 + i, 32 * mg + o1 + i)
                        nc.vector.scalar_tensor_tensor(
                            st32[:, o0:o1, 0:1], x8_sb[:, rows, 1:2],
                            wmat[:, 3 * i:3 * i + 1], st32[:, o0:o1, 0:1],
                            ALU.mult, ALU.add,
                        )
                        nc.vector.scalar_tensor_tensor(
                            st32[:, o0:o1, 255:256], x8_sb[:, rows, 254:255],
                            wmat[:, 3 * i + 2:3 * i + 3],
                            st32[:, o0:o1, 255:256],
                            ALU.mult, ALU.add,
                        )
                if mg < 3:
                    nc.sync.dma_start(out=out_d[:, 32 * mg:32 * mg + 32, :],
                                      in_=st32[0:64])
                    nc.sync.dma_start(
                        out=out_d[:, 128 + 32 * mg:128 + 32 * mg + 32, :],
                        in_=st32[64:128])
                else:
                    # PE rows (16:32) finish first; store them first
                    nc.sync.dma_start(out=out_d[:, 112:128, :],
                                      in_=st32[0:64, 16:32, :])
                    nc.sync.dma_start(out=out_d[:, 240:256, :],
                                      in_=st32[64:128, 16:32, :])
                    nc.sync.dma_start(out=out_d[:, 96:112, :],
                                      in_=st32[0:64, 0:16, :])
                    nc.sync.dma_start(out=out_d[:, 224:240, :],
                                      in_=st32[64:128, 0:16, :])

    nc.compile()
    return nc


def _host_consts(conv_w, bn_gamma, bn_beta, bn_mean, bn_var, lamb_l, lamb_h,
                 inside_all):
    import ml_dtypes

    f = np.float32
    eps = 1e-5
    bns = (bn_gamma / np.sqrt(bn_var + eps)).astype(f)          # (72,)
    bnb = (bn_beta - bn_mean * bns).astype(f)
    g = np.arange(128) % 64 // 8                                 # group of p
    g72 = np.zeros((72, 128), f)
    for p in range(128):
        for k in range(72):
            if k // 9 == g[p]:
                g72[k, p] = 1.0
    mask9 = np.zeros((72, K2), f)
    mask9[np.arange(72), np.arange(72) % 9] = 1.0
    ppool = np.zeros((128, 128), f)
    for m in range(128):
        ppool[m % 64, m] = 1.0 / 65536.0
        ppool[64 + m % 64, m] = 1.0 / 65536.0
    ia = inside_all.reshape(-1).astype(f)                        # (64,)
    ll = lamb_l.astype(f)
    lh = lamb_h.astype(f)
    a64 = (ll * (1.0 + ia)).astype(f)
    b64 = (1.0 + lh).astype(f)
    cl64 = (-ia * ll).astype(f)
    dup = lambda v: np.concatenate([v, v]).astype(f)
    cblob = np.zeros((128, CB_COLS), f)
    cblob[:, CB_PPOOL:CB_PPOOL + 128] = ppool
    cblob[0:72, CB_G72:CB_G72 + 128] = g72
    cblob[0:64, CB_CWT:CB_CWT + 72] = np.ascontiguousarray(conv_w.T.astype(f))
    cblob[0:72, CB_MASK9:CB_MASK9 + K2] = mask9
    cblob[0:72, CB_BNS] = bns
    cblob[0:72, CB_BNB] = bnb
    cblob[:, CB_AVEC] = dup(a64)
    cblob[:, CB_BVEC] = dup(b64)
    cblob[:, CB_CLVEC] = dup(cl64)
    return dict(
        cblob=cblob,
        i128=np.eye(128, dtype=ml_dtypes.bfloat16),
        i128_8=np.eye(128, dtype=ml_dtypes.float8_e4m3),
    )


def kernel(x, conv_w, bn_gamma, bn_beta, bn_mean, bn_var, lamb_l, lamb_h,
           inside_all):
    import ml_dtypes

    xb = np.asarray(x, np.float32).astype(ml_dtypes.bfloat16)
    x8 = np.asarray(x, np.float32).astype(ml_dtypes.float8_e4m3)
    consts = _host_consts(
        np.asarray(conv_w, np.float32), np.asarray(bn_gamma, np.float32),
        np.asarray(bn_beta, np.float32), np.asarray(bn_mean, np.float32),
        np.asarray(bn_var, np.float32), np.asarray(lamb_l, np.float32),
        np.asarray(lamb_h, np.float32), np.asarray(inside_all, np.float32),
    )
    nc = _build_program()
    in_maps = [
        dict(x=np.ascontiguousarray(xb[i]), x8=np.ascontiguousarray(x8[i]),
             **consts)
        for i in range(NCORES)
    ]
    trace = bool(os.environ.get("BASS_TRACE_KERNEL"))
    if trace:
        _install_ntff_hook()
    res = run_bass_kernel_spmd(
        nc, in_maps, core_ids=list(range(NCORES)), trace=trace
    )
    LAST_RESULT["exec_time_ns"] = res.exec_time_ns
    LAST_RESULT["raw"] = res
    return np.stack(
        [res.results[i]["out"].astype(np.float32) for i in range(NCORES)], axis=0
    )
